# revision 1
# baseline (speedup 1.0000x reference)
"""Trainium2 Bass kernel for nn_DependencyEncoder (shift-reduce tree-LSTM encoder).

Structure exploited: the transition sequence strictly alternates
shift (odd steps) / reduce (even steps), so the parser stack depth
oscillates 2->3->2...  Consequently:
  stack[0] is frozen at token0 forever,
  stack[1] holds a single running composed value v,
  each shifted token is consumed by the immediately following reduce.
The whole module collapses to 63 iterations of:
  shift  t=2k : tracking LSTM on x=[tok_k, v, tok_0]
  reduce t=2k+1: tracking LSTM on x=[tok_{k+1}, tok_k, v], then
                 head = tok_k (left-arc) or v (right-arc),
                 gates = W_{left/right} @ head + W_c @ th,  v <- (h_j, c_j)
Output = v_h after the last pair.  (Validated vs reference in numpy.)

Sharding: pure batch data-parallel, 384 rows -> 8 cores x 48 rows.
Per core (batch B=48):
  - activations are the PE stationary operand ("fm" feature-major layout,
    feature chunks 128/128/44 on partitions, batch on free), the big weight
    matrices are the streamed operand -> no weight reloads ever;
  - all matmul operands are float32r: full-rate fp32 PE mode (measured
    ~1.6e-4 rel err per matmul; exact fp32 streams at 1/4 rate).  float32r
    matmuls use the whole array (no column tiling), so gates are computed in
    one partition group, batch-major [48, 1500];
  - tanh is computed as 2*sigmoid(2x)-1 (u/g weight rows pre-scaled by 2 on
    the host) so every transcendental is a single Sigmoid activation op;
  - left/right arc selection is done by masking the stationary operand
    (h_l = token*mask_l, h_r = v*mask_r) and accumulating both W_left and
    W_right streams into the same PSUM.
"""
import numpy as np

import concourse.bacc as bacc
import concourse.bass as bass
import concourse.mybir as mybir
import concourse.tile as tile
from concourse.alu_op_type import AluOpType as alu
from concourse.bass import AP
from concourse.bass_utils import run_bass_kernel_spmd
from concourse.masks import make_identity

AF = mybir.ActivationFunctionType
f32 = mybir.dt.float32
f32r = mybir.dt.float32r

B_FULL, L, H, TD = 384, 64, 300, 64
NCORES = 8
B = B_FULL // NCORES          # 48 rows per core
K = int(__import__("os").environ.get("KERNEL_PAIRS", L - 1))   # 63 pairs
CH = [(0, 128), (128, 128), (256, 44)]   # feature chunks (offset, size)
NGATE = 5 * H                 # 1500 composition gate columns
NCHUNKS = ((0, 512), (512, 1024), (1024, NGATE))


# --------------------------------------------------------------------------
# host-side weight preparation
# --------------------------------------------------------------------------
def _comp_rhs(Wmat):
    """[5H, Kin] -> streamed rhs [Kin, 1500] with gate blocks reordered to
    (i, fh, fc, u, o) so sigmoid(o) can run as a separate parallel op;
    u-gate rows pre-scaled by 2 for the tanh->sigmoid trick."""
    Wg = Wmat.astype(np.float32).reshape(5, H, -1)
    # (i, o, fh, fc, u) -> (fh, fc, i, u, o): forget gates first so their
    # sigmoid + sum run while the rest of the gates stream
    Wg = np.concatenate([Wg[2:3], Wg[3:4], Wg[0:1], 2.0 * Wg[4:5], Wg[1:2]],
                        axis=0)
    return np.ascontiguousarray(Wg.reshape(5 * H, -1).T)


def _chunkify(Wr):
    """[Kin<=300, C] -> [128, 3, C] zero-padded feature chunks."""
    out = np.zeros((128, 3, Wr.shape[1]), np.float32)
    for c, (off, sz) in enumerate(CH):
        out[:sz, c, :] = Wr[off:off + sz, :]
    return out


def _prep_host(inputs):
    W_c = np.asarray(inputs["W_c"], np.float32)
    Uh_w = np.asarray(inputs["Uh_w"], np.float32)
    Ul_w = np.asarray(inputs["Ul_w"], np.float32)
    Ur_w = np.asarray(inputs["Ur_w"], np.float32)
    W_ih = np.asarray(inputs["W_ih"], np.float32)
    W_hh = np.asarray(inputs["W_hh"], np.float32)

    wl = _chunkify(_comp_rhs(Uh_w + Ul_w))        # [128, 3, 1500]
    wr = _chunkify(_comp_rhs(Uh_w + Ur_w))
    wcc = _comp_rhs(W_c)                          # [64, 1500]

    # tracking: torch gate order (i,f,g,o) -> (i,f,o,g), g rows *2
    perm = np.concatenate([np.arange(0, 64), np.arange(64, 128),
                           np.arange(192, 256), np.arange(128, 192)])
    scl = np.ones(256, np.float32)
    scl[192:] = 2.0
    Wih_r = W_ih[perm, :] * scl[:, None]          # [256, 900]
    Whh_r = (W_hh[perm, :] * scl[:, None]).T.copy()   # [64, 256]
    wtrk = np.zeros((128, 9, 256), np.float32)
    for s in range(3):
        for c, (off, sz) in enumerate(CH):
            wtrk[:sz, s * 3 + c, :] = Wih_r[:, s * H + off: s * H + off + sz].T
    return dict(wl=wl, wr=wr, wcc=np.ascontiguousarray(wcc),
                wtrk=wtrk, whh=np.ascontiguousarray(Whh_r))


# --------------------------------------------------------------------------
# device program
# --------------------------------------------------------------------------
_CACHED_NC = None


def _build_nc():
    nc = bacc.Bacc("TRN2", target_bir_lowering=False)
    tokh_d = nc.dram_tensor("tokh", [128, L, 3, B], f32r, kind="ExternalInput")
    tokc_d = nc.dram_tensor("tokc", [L, B, H], f32, kind="ExternalInput")
    wl_d = nc.dram_tensor("wl", [128, 3, NGATE], f32r, kind="ExternalInput")
    wr_d = nc.dram_tensor("wr", [128, 3, NGATE], f32r, kind="ExternalInput")
    wcc_d = nc.dram_tensor("wcc", [TD, NGATE], f32r, kind="ExternalInput")
    wtrk_d = nc.dram_tensor("wtrk", [128, 9, 256], f32r, kind="ExternalInput")
    whh_d = nc.dram_tensor("whh", [TD, 256], f32r, kind="ExternalInput")
    mlr_d = nc.dram_tensor("mlr", [K, B], f32, kind="ExternalInput")
    mrr_d = nc.dram_tensor("mrr", [K, B], f32, kind="ExternalInput")
    mld_d = nc.dram_tensor("mld", [B, K], f32, kind="ExternalInput")
    mrd_d = nc.dram_tensor("mrd", [B, K], f32, kind="ExternalInput")
    th0t_d = nc.dram_tensor("th0t", [TD, B], f32r, kind="ExternalInput")
    tc0_d = nc.dram_tensor("tc0", [B, TD], f32, kind="ExternalInput")
    out_d = nc.dram_tensor("out", [3, 128, B], f32r, kind="ExternalOutput")

    with tile.TileContext(nc) as tc_:
        with (
            tc_.tile_pool(name="sg", bufs=1) as sg,
            tc_.tile_pool(name="rot", bufs=3) as rot,
            tc_.tile_pool(name="tkc", bufs=3) as tkc,
            tc_.tile_pool(name="st", bufs=6) as st,
            tc_.tile_pool(name="psA", bufs=3, space="PSUM") as psA,
            tc_.tile_pool(name="psT", bufs=2, space="PSUM") as psT,
            tc_.tile_pool(name="psC", bufs=1, space="PSUM") as psC,
        ):
            # ---------------- resident tiles ----------------
            tokh = sg.tile([128, L, 3, B], f32r)    # feature-major tokens (h)
            wl = sg.tile([128, 3, NGATE], f32r)
            wr = sg.tile([128, 3, NGATE], f32r)
            wcc = sg.tile([TD, NGATE], f32r)
            wtrk = sg.tile([128, 9, 256], f32r)
            whh = sg.tile([TD, 256], f32r)
            mlr = sg.tile([128, K, B], f32)         # left mask bcast over partitions
            mrr = sg.tile([128, K, B], f32)
            mld = sg.tile([B, K], f32)              # per-partition masks (batch rows)
            mrd = sg.tile([B, K], f32)
            th0t = sg.tile([TD, B], f32r)
            tc0s = sg.tile([B, TD], f32)
            ident = sg.tile([128, 128], f32)

            make_identity(nc, ident[:])

            # all layout transforms were done host-side: plain copies only.
            # tokens arrive in 8 groups so pair-0 compute starts early
            for gvii in range(8):
                gl = gvii * (L // 8)
                nc.sync.dma_start(tokh[:, gl:gl + L // 8, :, :],
                                  tokh_d[:, gl:gl + L // 8, :, :])
            nc.sync.dma_start(wl[:], wl_d[:])
            nc.sync.dma_start(wr[:], wr_d[:])
            nc.sync.dma_start(wcc[:], wcc_d[:])
            nc.sync.dma_start(wtrk[:], wtrk_d[:])
            nc.sync.dma_start(whh[:], whh_d[:])
            for dst, srcd in ((mlr, mlr_d), (mrr, mrr_d)):
                bsrc = AP(tensor=srcd, offset=0, ap=[[0, 128], [B, K], [1, B]])
                nc.sync.dma_start(dst[:], bsrc)
            nc.sync.dma_start(mld[:], mld_d[:])
            nc.sync.dma_start(mrd[:], mrd_d[:])
            nc.sync.dma_start(th0t[:], th0t_d[:])
            nc.sync.dma_start(tc0s[:], tc0_d[:])

            # composition psum: persistent, 3 banks
            cp = psC.tile([B, 1536], f32)

            # float32r matmuls: full-rate fp32 PE mode
            mm = nc.tensor.matmul

            def tok_ap(c, l):
                return tokh[:CH[c][1], l, c, :]

            def track_mms(bufs_l, s1_l, s2_l, thT_in=None):
                """Emit the 9 x-stream matmuls (+U if thT_in) for one step.
                Order (buf, s2, s1): the s1 operand is the freshest value, so
                it goes last in the PE's in-order queue."""
                ps = psA.tile([B, 256], f32, tag="trk")
                first = True
                for s, srcl in ((0, bufs_l), (2, s2_l), (1, s1_l)):
                    for c in range(3):
                        sz = CH[c][1]
                        mm(ps[:], srcl[c], wtrk[:sz, s * 3 + c, :],
                           start=first, stop=False)
                        first = False
                if thT_in is not None:
                    mm(ps[:], thT_in[:], whh[:], start=False, stop=True)
                return ps

            def track_tail(ps, tc_in):
                """Sigmoid + LSTM cell + transposed next-th for one step."""
                sa = rot.tile([B, 256], f32, tag="sa")
                nc.scalar.activation(sa[:], ps[:], AF.Sigmoid)
                d3 = st.tile([B, TD], f32, tag="d3")
                nc.vector.tensor_tensor(d3[:], sa[:, 64:128], tc_in[:], alu.mult)
                d1 = st.tile([B, TD], f32, tag="d1")
                nc.gpsimd.tensor_tensor(d1[:], sa[:, 0:64], sa[:, 192:256], alu.mult)
                d2 = st.tile([B, TD], f32, tag="d2")
                nc.vector.scalar_tensor_tensor(d2[:], d1[:], 2.0, sa[:, 0:64],
                                               alu.mult, alu.subtract)
                tc_o = st.tile([B, TD], f32, tag="tc")
                nc.vector.tensor_tensor(tc_o[:], d3[:], d2[:], alu.add)
                ptc = psT.tile([128, B], f32, tag="ptr")
                nc.tensor.transpose(ptc[0:TD, :], tc_o[:], ident[0:B, 0:B])
                pso = psT.tile([128, B], f32, tag="ptr")
                nc.tensor.transpose(pso[0:TD, :], sa[:, 128:192], ident[0:B, 0:B])
                sT = st.tile([TD, B], f32, tag="sT")
                nc.scalar.activation(sT[:], ptc[0:TD, :], AF.Sigmoid, scale=2.0)
                soT = st.tile([TD, B], f32, tag="soT")
                nc.vector.tensor_copy(soT[:], pso[0:TD, :])
                pp = st.tile([TD, B], f32, tag="pp")
                nc.vector.tensor_tensor(pp[:], sT[:], soT[:], alu.mult)
                thT_o = st.tile([TD, B], f32r, tag="thT")
                nc.vector.scalar_tensor_tensor(thT_o[:], pp[:], 2.0, soT[:],
                                               alu.mult, alu.subtract)
                return thT_o, tc_o

            thT_prev = th0t
            tc_prev = tc0s
            vh_prev = None          # fm chunks of running value v (h)
            vc_prev = None          # batch-major v (c) [48, 300]

            for k in range(K):
                mlr_k = mlr[:, k, :]
                mrr_k = mrr[:, k, :]
                mld_k = mld[:, k:k + 1]
                mrd_k = mrd[:, k:k + 1]

                if k == 0:
                    vh_l = [tok_ap(c, 0) for c in range(3)]
                else:
                    vh_l = [vh_prev[:CH[c][1], c, :] for c in range(3)]

                # ---- c tokens for this pair: streamed from DRAM
                tokc_t = tkc.tile([B, H], f32, tag="tokc")
                nc.sync.dma_start(tokc_t[:], tokc_d[k, :, :])

                # ---- head tiles for composition
                hl = rot.tile([128, 3, B], f32r, tag="hl")
                hr = rot.tile([128, 3, B], f32r, tag="hr")
                for c, (off, sz) in enumerate(CH):
                    nc.gpsimd.tensor_tensor(hl[:sz, c, :], tok_ap(c, k),
                                            mlr_k[:sz, :], alu.mult)
                    nc.gpsimd.tensor_tensor(hr[:sz, c, :], vh_l[c],
                                            mrr_k[:sz, :], alu.mult)
                ch1 = rot.tile([B, H], f32, tag="ch1")
                nc.gpsimd.tensor_scalar(ch1[:], tokc_t[:], mld_k, None, alu.mult)
                ch = rot.tile([B, H], f32, tag="ch")
                if k == 0:
                    nc.vector.scalar_tensor_tensor(ch[:], tokc_t[:], mrd_k,
                                                   ch1[:], alu.mult, alu.add)
                else:
                    nc.vector.scalar_tensor_tensor(ch[:], vc_prev[:], mrd_k,
                                                   ch1[:], alu.mult, alu.add)

                # ---- matmul emission order = PE in-order queue.
                # track-a streams first (its U uses thT_prev: no stall), then
                # track-b x-streams, then its U (waits thT_a), then the fat
                # composition streams overlapping the track-b tail.
                toks_k = [tok_ap(c, k) for c in range(3)]
                toks_k1 = [tok_ap(c, k + 1) for c in range(3)]
                toks_0 = [tok_ap(c, 0) for c in range(3)]
                ps_a = track_mms(toks_k, vh_l, toks_0, thT_in=thT_prev)
                thT_a, tc_a = track_tail(ps_a, tc_prev)
                ps_b = track_mms(toks_k1, toks_k, vh_l)
                mm(ps_b[:], thT_a[:], whh[:], start=False, stop=True)
                thT_b, tc_b = track_tail(ps_b, tc_a)

                # ---- composition matmuls (heads; W_c accumulated after track)
                for nlo, nhi in NCHUNKS:
                    for c, (off, sz) in enumerate(CH):
                        mm(cp[:, nlo:nhi], hl[:sz, c, :], wl[:sz, c, nlo:nhi],
                           start=(c == 0), stop=False)
                    for c, (off, sz) in enumerate(CH):
                        mm(cp[:, nlo:nhi], hr[:sz, c, :], wr[:sz, c, nlo:nhi],
                           start=False, stop=False)

                # ---- W_c stream into composition psum
                for nlo, nhi in NCHUNKS:
                    mm(cp[:, nlo:nhi], thT_b[:], wcc[:, nlo:nhi],
                       start=False, stop=True)

                # ---- composition elementwise, batch-major [48, 300] slices
                sc = rot.tile([B, NGATE], f32, tag="sc")
                nc.scalar.activation(sc[:, 0:600], cp[:, 0:600], AF.Sigmoid)
                nc.scalar.activation(sc[:, 600:1200], cp[:, 600:1200],
                                     AF.Sigmoid)
                nc.scalar.activation(sc[:, 1200:1500], cp[:, 1200:1500],
                                     AF.Sigmoid)
                SCfh = sc[:, 0:300]
                SCfc = sc[:, 300:600]
                SCi = sc[:, 600:900]
                SCu = sc[:, 900:1200]
                SCo = sc[:, 1200:1500]
                t2 = rot.tile([B, H], f32, tag="t2")
                nc.gpsimd.tensor_tensor(t2[:], SCfh, SCfc, alu.add)
                t3 = rot.tile([B, H], f32, tag="t3")
                nc.gpsimd.tensor_tensor(t3[:], t2[:], ch[:], alu.mult)
                pu = rot.tile([B, H], f32, tag="pu")
                nc.vector.tensor_tensor(pu[:], SCi, SCu, alu.mult)
                xu = rot.tile([B, H], f32, tag="xu")
                nc.vector.scalar_tensor_tensor(xu[:], pu[:], 2.0, SCi,
                                               alu.mult, alu.subtract)
                c_j = rot.tile([B, H], f32, tag="vc")
                nc.vector.tensor_tensor(c_j[:], xu[:], t3[:], alu.add)
                scj = rot.tile([B, H], f32, tag="scj")
                nc.scalar.activation(scj[:], c_j[:], AF.Sigmoid, scale=2.0)
                # h_j, its transpose and the feature-major copy are emitted
                # per feature chunk so the next pair's v-dependent matmuls can
                # start as soon as their chunk lands
                h_j = rot.tile([B, H], f32, tag="hj")
                vh = rot.tile([128, 3, B], f32r, tag="vh")
                copy_eng = (nc.vector.tensor_copy,
                            lambda o, i: nc.scalar.activation(o, i, AF.Copy),
                            nc.vector.tensor_copy)
                for c, (off, sz) in enumerate(CH):
                    qq = rot.tile([B, 128], f32, tag="qq")
                    nc.vector.tensor_tensor(qq[:, :sz], scj[:, off:off + sz],
                                            SCo[:, off:off + sz], alu.mult)
                    nc.vector.scalar_tensor_tensor(
                        h_j[:, off:off + sz], qq[:, :sz], 2.0,
                        SCo[:, off:off + sz], alu.mult, alu.subtract)
                    pc = psT.tile([128, B], f32, tag="ptr")
                    nc.tensor.transpose(pc[0:sz, :], h_j[:, off:off + sz],
                                        ident[0:B, 0:B])
                    copy_eng[c](vh[:sz, c, :], pc[0:sz, :])

                vh_prev, vc_prev = vh, c_j
                thT_prev, tc_prev = thT_b, tc_b

            # ---- output: v_h in feature-major chunk layout [3, 128, B]
            for c in range(3):
                nc.sync.dma_start(out_d[c, :, :], vh_prev[:, c, :])

    nc.compile()
    return nc


def _get_nc():
    global _CACHED_NC
    if _CACHED_NC is None:
        _CACHED_NC = _build_nc()
    return _CACHED_NC


def make_in_maps(inputs):
    """Build the 8 per-core input maps from the full-problem inputs."""
    seq = np.asarray(inputs["sequence"], np.float32)
    tr = np.asarray(inputs["transitions"])
    th0 = np.asarray(inputs["th0"], np.float32)
    tc0 = np.asarray(inputs["tc0"], np.float32)
    wts = _prep_host(inputs)

    in_maps = []
    for i in range(NCORES):
        s = slice(i * B, (i + 1) * B)
        sq = seq[s]                                  # [B, L, 600]
        # feature-major h tokens [128, L, 3, B]
        tokh = np.zeros((128, L, 3, B), np.float32)
        for c, (off, sz) in enumerate(CH):
            tokh[:sz, :, c] = sq[:, :, off:off + sz].transpose(2, 1, 0)
        # c tokens, token-major for per-pair streaming
        tokc = np.ascontiguousarray(sq[:, :, H:].transpose(1, 0, 2))  # [L,B,H]

        is_left = (tr[s, 1::2].T == 2).astype(np.float32)[:K]   # [K, B]
        in_maps.append(dict(
            tokh=tokh, tokc=tokc,
            wl=wts["wl"], wr=wts["wr"], wcc=wts["wcc"],
            wtrk=wts["wtrk"], whh=wts["whh"],
            mlr=np.ascontiguousarray(is_left),
            mrr=np.ascontiguousarray(1.0 - is_left),
            mld=np.ascontiguousarray(is_left.T),
            mrd=np.ascontiguousarray(1.0 - is_left.T),
            th0t=np.ascontiguousarray(th0[s].T),
            tc0=np.ascontiguousarray(tc0[s]),
        ))
    return in_maps


def assemble_out(res_list):
    """Per-core [3, 128, B] chunk outputs -> [B_full, 300] float32."""
    outs = []
    for r in res_list:
        arr = r["out"]                       # [3, 128, B]
        o = np.empty((B, H), np.float32)
        for c, (off, sz) in enumerate(CH):
            o[:, off:off + sz] = arr[c, :sz, :].T
        outs.append(o)
    return np.concatenate(outs, axis=0)


def kernel(**inputs) -> np.ndarray:
    nc = _get_nc()
    in_maps = make_in_maps(inputs)
    res = run_bass_kernel_spmd(nc, in_maps, core_ids=list(range(NCORES)))
    return assemble_out(res.results)



# revision 4
# speedup vs baseline: 1.3314x; 1.3314x over previous
"""Trainium2 Bass kernel for nn_DependencyEncoder (shift-reduce tree-LSTM).

Structure exploited (validated vs reference): transitions strictly alternate
shift/reduce, so the stack collapses to [tok0, v] and the module becomes 63
iterations of two tracking-LSTM steps plus one dependency composition:
  pair k:  track a on x=[tok_k, v, tok_0]
           track b on x=[tok_{k+1}, tok_k, v]
           head = tok_k (left arc) or v (right arc)
           gates = W_{l/r} @ head + W_c @ th_b ; v <- (h_j, c_j)
Output = v_h after the last pair.

Layout strategy (differs from the act-stationary baseline): everything is
GATE-MAJOR / WEIGHT-STATIONARY.  Weight tiles [K_feat<=128, M_gate<=128] are
the PE stationary operand; the moving operand is the activation [K_feat, B=48]
in fp16 (1 cycle/row in the cost model at any moving size, vs 4x penalty for
f32r under 256 rows).  Outputs land feature/gate-major [gate, batch] in PSUM,
so the LSTM elementwise runs directly on partition-aligned tiles and the next
iteration's matmul operands come out in the right orientation: NO transposes
anywhere in the loop.

Per iteration the PE sees only ~185 small matmuls (48-wide moving), the
sigmoid work is 6 Activation ops, and the cell math is ~20 DVE/Pool ops.
The tracking gates live on 64 partitions x 4 slots (i,f,2g,o) so every
cross-gate elementwise op is partition-aligned; composition gates live in a
[128, 15, 64] PSUM tile, slots (gate-block, H-chunk), H-chunks aligned with
the feature chunks of v / c_head.

Sharding: pure batch data-parallel, 384 rows -> 8 cores x 48 rows.
Masking (left/right arc) for tokens is folded on the host; only the v-side
masks run on-device.
"""
import numpy as np

import concourse.bacc as bacc
import concourse.bass as bass
import concourse.mybir as mybir
import concourse.tile as tile
from concourse.alu_op_type import AluOpType as alu
from concourse.bass import AP
from concourse.bass_utils import run_bass_kernel_spmd

AF = mybir.ActivationFunctionType
f32 = mybir.dt.float32
f16 = mybir.dt.float16

B_FULL, L, H, TD = 384, 64, 300, 64
NCORES = 8
B = B_FULL // NCORES          # 48 rows per core
K = L - 1                     # 63 pairs
CH = [(0, 128), (128, 128), (256, 44)]   # H chunks (offset, size)
NG = 4 * TD                   # 256 tracking gate columns


# --------------------------------------------------------------------------
# host-side weight preparation
# --------------------------------------------------------------------------
def _comp_wt(Wmat):
    """[5H, Kin] -> weight tiles [128, 3ci, 5g, 3co, 128] fp16.
    Gate-block order (fh, fc, i, 2u, o); u rows pre-scaled by 2 so
    tanh(u) = 2*sigmoid(2u) - 1 costs one sigmoid."""
    Wg = Wmat.astype(np.float32).reshape(5, H, -1)    # (i, o, fh, fc, u)
    Wg = np.stack([Wg[2], Wg[3], Wg[0], 2.0 * Wg[4], Wg[1]], axis=0)
    kin = Wmat.shape[1]
    out = np.zeros((128, 3, 5, 3, 128), np.float16)
    for ci, (offi, szi) in enumerate(CH):
        if offi >= kin:
            continue
        szi_eff = min(szi, kin - offi)
        for g in range(5):
            for co, (offo, szo) in enumerate(CH):
                blk = Wg[g, offo:offo + szo, offi:offi + szi_eff]   # [szo, szi]
                out[:szi_eff, ci, g, co, :szo] = blk.T
    return out


def _trk_wt(W_ih, W_hh):
    """Tracking weights -> [128, 3s, 3ci, 4slot, 64] fp16 and U [64, 4, 64].
    Slot order (i, f, 2g, o): torch rows (i, f, g, o) with g rows doubled."""
    Wih = W_ih.astype(np.float32)        # [256, 900]
    Whh = W_hh.astype(np.float32)        # [256, 64]
    scl = np.ones((256, 1), np.float32)
    scl[128:192] = 2.0                   # g rows
    Wih = Wih * scl
    Whh = Whh * scl
    wt = np.zeros((128, 3, 3, 4, 64), np.float16)
    for s in range(3):
        for ci, (offi, szi) in enumerate(CH):
            for slot in range(4):
                rows = Wih[slot * 64:(slot + 1) * 64,
                           s * H + offi: s * H + offi + szi]   # [64, szi]
                wt[:szi, s, ci, slot, :] = rows.T
    wu = np.zeros((64, 4, 64), np.float16)
    for slot in range(4):
        wu[:, slot, :] = Whh[slot * 64:(slot + 1) * 64, :].T
    return wt, wu


def _prep_host(inputs):
    W_c = np.asarray(inputs["W_c"], np.float32)
    Uh_w = np.asarray(inputs["Uh_w"], np.float32)
    Ul_w = np.asarray(inputs["Ul_w"], np.float32)
    Ur_w = np.asarray(inputs["Ur_w"], np.float32)
    wl = _comp_wt(Uh_w + Ul_w)
    wr = _comp_wt(Uh_w + Ur_w)
    wcc_full = _comp_wt(W_c)             # Kin=64 -> only ci=0 rows used
    wcc = np.ascontiguousarray(wcc_full[:64, 0])     # [64, 5, 3, 128]
    wtrk, whh = _trk_wt(np.asarray(inputs["W_ih"]), np.asarray(inputs["W_hh"]))
    return dict(wl=wl, wr=wr, wcc=wcc, wtrk=wtrk, whh=whh)


# --------------------------------------------------------------------------
# device program
# --------------------------------------------------------------------------
_CACHED_NC = None


def _build_nc():
    nc = bacc.Bacc("TRN2", target_bir_lowering=False)
    tokh_d = nc.dram_tensor("tokh", [128, L, 3, B], f16, kind="ExternalInput")
    tokl_d = nc.dram_tensor("tokl", [128, K, 3, B], f16, kind="ExternalInput")
    tokcl_d = nc.dram_tensor("tokcl", [128, K, 3, B], f32, kind="ExternalInput")
    tokc0_d = nc.dram_tensor("tokc0", [128, 3, B], f32, kind="ExternalInput")
    wl_d = nc.dram_tensor("wl", [128, 3, 5, 3, 128], f16, kind="ExternalInput")
    wr_d = nc.dram_tensor("wr", [128, 3, 5, 3, 128], f16, kind="ExternalInput")
    wcc_d = nc.dram_tensor("wcc", [64, 5, 3, 128], f16, kind="ExternalInput")
    wtrk_d = nc.dram_tensor("wtrk", [128, 3, 3, 4, 64], f16, kind="ExternalInput")
    whh_d = nc.dram_tensor("whh", [64, 4, 64], f16, kind="ExternalInput")
    mrf_d = nc.dram_tensor("mrf", [K, B], f32, kind="ExternalInput")
    mrh_d = nc.dram_tensor("mrh", [K, B], f16, kind="ExternalInput")
    th0_d = nc.dram_tensor("th0", [TD, B], f16, kind="ExternalInput")
    tc0_d = nc.dram_tensor("tc0", [TD, B], f32, kind="ExternalInput")
    out_d = nc.dram_tensor("out", [128, 3, B], f32, kind="ExternalOutput")

    mm = None  # set below

    with tile.TileContext(nc) as tc_:
        with (
            tc_.tile_pool(name="sg", bufs=1) as sg,
            tc_.tile_pool(name="st", bufs=2) as st,
            tc_.tile_pool(name="rot", bufs=2) as rot,
            tc_.tile_pool(name="psT", bufs=2, space="PSUM") as psT,
            tc_.tile_pool(name="psC", bufs=2, space="PSUM") as psC,
        ):
            # ---------------- resident tiles ----------------
            tokh = sg.tile([128, L, 3, B], f16)
            tokl = sg.tile([128, K, 3, B], f16)
            tokcl = sg.tile([128, K, 3, B], f32)
            tokc0 = sg.tile([128, 3, B], f32)
            wl = sg.tile([128, 3, 5, 3, 128], f16)
            wr = sg.tile([128, 3, 5, 3, 128], f16)
            wcc = sg.tile([64, 5, 3, 128], f16)
            wtrk = sg.tile([128, 3, 3, 4, 64], f16)
            whh = sg.tile([64, 4, 64], f16)
            mrf = sg.tile([128, K, B], f32)
            mrh = sg.tile([128, K, B], f16)
            th0 = sg.tile([TD, B], f16)
            tc0 = sg.tile([TD, B], f32)

            # weights + state first, then tokens in groups so iter-0 deps
            # land early
            nc.sync.dma_start(wl[:], wl_d[:])
            nc.sync.dma_start(wr[:], wr_d[:])
            nc.sync.dma_start(wtrk[:], wtrk_d[:])
            nc.sync.dma_start(whh[:], whh_d[:])
            nc.sync.dma_start(wcc[:], wcc_d[:])
            for dst, srcd in ((mrf, mrf_d), (mrh, mrh_d)):
                bsrc = AP(tensor=srcd, offset=0, ap=[[0, 128], [B, K], [1, B]])
                nc.sync.dma_start(dst[:], bsrc)
            nc.sync.dma_start(th0[:], th0_d[:])
            nc.sync.dma_start(tc0[:], tc0_d[:])
            nc.sync.dma_start(tokc0[:], tokc0_d[:])
            NGRP = 8
            for g in range(NGRP):
                lo = g * (L // NGRP)
                hi = lo + (L // NGRP)
                nc.sync.dma_start(tokh[:, lo:hi], tokh_d[:, lo:hi])
                khi = min(hi, K)
                if lo < K:
                    nc.sync.dma_start(tokl[:, lo:khi], tokl_d[:, lo:khi])
                    nc.sync.dma_start(tokcl[:, lo:khi], tokcl_d[:, lo:khi])

            mm = nc.tensor.matmul

            # zero the never-written partition rows (44:128) of both comp
            # psum buffers once, so sigmoid reads are finite there forever
            for _b in range(2):
                cpz = psC.tile([128, 15, 64], f32, tag="cp")
                nc.vector.memset(cpz[44:128, :, :], 0.0)

            def mask_h(k):
                return mrh[:, k, :].unsqueeze(1).broadcast_to((128, 3, B))

            def mask_f(k):
                return mrf[:, k, :].unsqueeze(1).broadcast_to((128, 3, B))

            # ---- initial state: v = tok0, hr_0 = mr_0*tok0, ch_0 ----
            vb = tokh[:, 0]              # [128, 3, B] fp16 (h part of v)
            vc = tokc0                   # [128, 3, B] f32  (c part of v)
            hr = rot.tile([128, 3, B], f16, tag="hr")
            nc.vector.tensor_tensor(hr[:], tokh[:, 0], mask_h(0), alu.mult)
            mrvc = rot.tile([128, 3, B], f32, tag="mrvc")
            nc.gpsimd.tensor_tensor(mrvc[:], tokc0[:], mask_f(0), alu.mult)
            ch = rot.tile([128, 3, B], f32, tag="ch")
            nc.gpsimd.tensor_tensor(ch[:], tokcl[:, 0], mrvc[:], alu.add)
            thb = th0
            tc_st = tc0

            def track_mms(ps, tok_blocks, v_in, th_in):
                """Emit one tracking step's matmuls into psum ps [64, 4, 48].
                tok_blocks: list of (block_idx, token_ap_3chunk) emitted
                before the v part; v enters as block v_blk."""
                for slot in range(4):
                    mm(ps[:, slot, :], whh[:, slot, :], th_in[:],
                       start=True, stop=False)
                for blk, tok in tok_blocks:
                    for slot in range(4):
                        for ci, (offi, szi) in enumerate(CH):
                            mm(ps[:, slot, :], wtrk[:szi, blk, ci, slot, :],
                               tok[:szi, ci, :], start=False, stop=False)
                return ps

            def track_v_mms(ps, v_blk, v_in):
                for slot in range(4):
                    for ci, (offi, szi) in enumerate(CH):
                        mm(ps[:, slot, :], wtrk[:szi, v_blk, ci, slot, :],
                           v_in[:szi, ci, :], start=False,
                           stop=(ci == 2))
                return ps

            def track_tail(ps, tc_in):
                """sigmoid + LSTM cell; returns (thb fp16 [64,48], tc f32)."""
                sa = st.tile([64, 4, B], f32, tag="sa")
                nc.scalar.activation(sa[:], ps[:], AF.Sigmoid)
                si, sf, sgg, so = (sa[:, 0, :], sa[:, 1, :],
                                   sa[:, 2, :], sa[:, 3, :])
                p = st.tile([64, B], f32, tag="p")
                nc.vector.tensor_tensor(p[:], si, sgg, alu.mult)
                r = st.tile([64, B], f32, tag="r")
                nc.gpsimd.tensor_tensor(r[:], sf, tc_in[:], alu.mult)
                q = st.tile([64, B], f32, tag="q")
                nc.vector.scalar_tensor_tensor(q[:], p[:], 2.0, si,
                                               alu.mult, alu.subtract)
                tco = st.tile([64, B], f32, tag="tc")
                nc.vector.tensor_tensor(tco[:], q[:], r[:], alu.add)
                u = st.tile([64, B], f32, tag="u")
                nc.scalar.activation(u[:], tco[:], AF.Sigmoid, scale=2.0)
                m = st.tile([64, B], f32, tag="m")
                nc.vector.tensor_tensor(m[:], u[:], so, alu.mult)
                tho = st.tile([64, B], f16, tag="th")
                nc.vector.scalar_tensor_tensor(tho[:], m[:], 2.0, so,
                                               alu.mult, alu.subtract)
                return tho, tco

            for k in range(K):
                # ---------- PE: ordered so stalling mms come late ----------
                ta = psT.tile([64, 4, B], f32, tag="ta")
                tb = psT.tile([64, 4, B], f32, tag="ta")
                cp = psC.tile([128, 15, 64], f32, tag="cp")

                # step a: U first (th ready), then token blocks
                track_mms(ta, [(0, tokh[:, k]), (2, tokh[:, 0])], None, thb)
                # step b token parts (U comes after th_a)
                for slot in range(4):
                    for blk, tok in ((0, tokh[:, k + 1]), (1, tokh[:, k])):
                        for ci, (offi, szi) in enumerate(CH):
                            mm(tb[:, slot, :], wtrk[:szi, blk, ci, slot, :],
                               tok[:szi, ci, :],
                               start=(blk == 0 and ci == 0), stop=False)
                # step a v part (stalls until v ready; ahead of comp-l so the
                # sigmoid_a chain isn't queued behind 45 composition matmuls)
                track_v_mms(ta, 1, vb)
                # comp left (host-masked tokens; fills PE during track-a tail)
                for g in range(5):
                    for co, (offo, szo) in enumerate(CH):
                        slot = g * 3 + co
                        for ci, (offi, szi) in enumerate(CH):
                            mm(cp[:szo, slot, :B], wl[:szi, ci, g, co, :szo],
                               tokl[:szi, k, ci, :],
                               start=(ci == 0), stop=False)
                # step b v part
                for slot in range(4):
                    for ci, (offi, szi) in enumerate(CH):
                        mm(tb[:, slot, :], wtrk[:szi, 2, ci, slot, :],
                           vb[:szi, ci, :], start=False, stop=False)
                # comp right (stalls until hr ready)
                for g in range(5):
                    for co, (offo, szo) in enumerate(CH):
                        slot = g * 3 + co
                        for ci, (offi, szi) in enumerate(CH):
                            mm(cp[:szo, slot, :B], wr[:szi, ci, g, co, :szo],
                               hr[:szi, ci, :], start=False, stop=False)

                # ---------- track a tail ----------
                thb_a, tc_a = track_tail(ta, tc_st)

                # step b U (waits th_a)
                for slot in range(4):
                    mm(tb[:, slot, :], whh[:, slot, :], thb_a[:],
                       start=False, stop=True)

                # ---------- track b tail ----------
                thb_b, tc_b = track_tail(tb, tc_a)

                # comp W_c (waits th_b)
                for g in range(5):
                    for co, (offo, szo) in enumerate(CH):
                        slot = g * 3 + co
                        mm(cp[:szo, slot, :B], wcc[:, g, co, :szo],
                           thb_b[:], start=False, stop=True)

                # ---------- composition tail ----------
                sc = rot.tile([128, 15, B], f32, tag="sc")
                nc.scalar.activation(sc[:], cp[:, :, :B], AF.Sigmoid)
                SFH = sc[:, 0:3, :]
                SFC = sc[:, 3:6, :]
                SI = sc[:, 6:9, :]
                SU = sc[:, 9:12, :]
                SO = sc[:, 12:15, :]
                s2 = rot.tile([128, 3, B], f32, tag="s2")
                nc.vector.tensor_tensor(s2[:], SFH, SFC, alu.add)
                t3 = rot.tile([128, 3, B], f32, tag="t3")
                nc.vector.tensor_tensor(t3[:], s2[:], ch[:], alu.mult)
                pu = rot.tile([128, 3, B], f32, tag="pu")
                nc.gpsimd.tensor_tensor(pu[:], SI, SU, alu.mult)
                xu = rot.tile([128, 3, B], f32, tag="xu")
                nc.vector.scalar_tensor_tensor(xu[:], pu[:], 2.0, SI,
                                               alu.mult, alu.subtract)
                cj = rot.tile([128, 3, B], f32, tag="vc")
                nc.vector.tensor_tensor(cj[:], xu[:], t3[:], alu.add)
                scj = rot.tile([128, 3, B], f32, tag="scj")
                nc.scalar.activation(scj[:], cj[:], AF.Sigmoid, scale=2.0)
                qq = rot.tile([128, 3, B], f32, tag="qq")
                nc.vector.tensor_tensor(qq[:], scj[:], SO, alu.mult)
                hj = rot.tile([128, 3, B], f16, tag="vb")
                nc.vector.scalar_tensor_tensor(hj[:], qq[:], 2.0, SO,
                                               alu.mult, alu.subtract)
                if k + 1 < K:
                    hr = rot.tile([128, 3, B], f16, tag="hr")
                    nc.vector.tensor_tensor(hr[:], hj[:], mask_h(k + 1),
                                            alu.mult)
                    mrvc = rot.tile([128, 3, B], f32, tag="mrvc")
                    nc.gpsimd.tensor_tensor(mrvc[:], cj[:], mask_f(k + 1),
                                            alu.mult)
                    ch = rot.tile([128, 3, B], f32, tag="ch")
                    nc.gpsimd.tensor_tensor(ch[:], tokcl[:, k + 1], mrvc[:],
                                            alu.add)
                else:
                    # final output in f32
                    hjf = rot.tile([128, 3, B], f32, tag="hjf")
                    nc.vector.scalar_tensor_tensor(hjf[:], qq[:], 2.0, SO,
                                                   alu.mult, alu.subtract)
                    nc.sync.dma_start(out_d[:], hjf[:])

                vb = hj
                vc = cj
                thb = thb_b
                tc_st = tc_b

    nc.compile()
    return nc


def _get_nc():
    global _CACHED_NC
    if _CACHED_NC is None:
        _CACHED_NC = _build_nc()
    return _CACHED_NC


# --------------------------------------------------------------------------
# host-side input marshaling
# --------------------------------------------------------------------------
def make_in_maps(inputs):
    seq = np.asarray(inputs["sequence"], np.float32)
    tr = np.asarray(inputs["transitions"])
    th0 = np.asarray(inputs["th0"], np.float32)
    tc0 = np.asarray(inputs["tc0"], np.float32)
    wts = _prep_host(inputs)

    in_maps = []
    for i in range(NCORES):
        s = slice(i * B, (i + 1) * B)
        sq = seq[s]                                  # [B, L, 600]
        toh = sq[:, :, :H]                           # [B, L, 300]
        toc = sq[:, :, H:]
        is_left = (tr[s, 1::2].T == 2).astype(np.float32)[:K]   # [K, B]
        mr = 1.0 - is_left                                       # right mask

        tokh = np.zeros((128, L, 3, B), np.float16)
        tokl = np.zeros((128, K, 3, B), np.float16)
        tokcl = np.zeros((128, K, 3, B), np.float32)
        for c, (off, sz) in enumerate(CH):
            tokh[:sz, :, c] = toh[:, :, off:off + sz].transpose(2, 1, 0)
            tokl[:sz, :, c] = (toh[:, :K, off:off + sz]
                               * is_left.T[:, :, None]).transpose(2, 1, 0)
            tokcl[:sz, :, c] = (toc[:, :K, off:off + sz]
                                * is_left.T[:, :, None]).transpose(2, 1, 0)
        tokc0 = np.zeros((128, 3, B), np.float32)
        for c, (off, sz) in enumerate(CH):
            tokc0[:sz, c] = toc[:, 0, off:off + sz].T

        in_maps.append(dict(
            tokh=tokh, tokl=tokl, tokcl=tokcl, tokc0=tokc0,
            wl=wts["wl"], wr=wts["wr"], wcc=wts["wcc"],
            wtrk=wts["wtrk"], whh=wts["whh"],
            mrf=np.ascontiguousarray(mr),
            mrh=np.ascontiguousarray(mr.astype(np.float16)),
            th0=np.ascontiguousarray(th0[s].T.astype(np.float16)),
            tc0=np.ascontiguousarray(tc0[s].T),
        ))
    return in_maps


def assemble_out(res_list):
    outs = []
    for r in res_list:
        arr = r["out"]                       # [128, 3, B]
        o = np.empty((B, H), np.float32)
        for c, (off, sz) in enumerate(CH):
            o[:, off:off + sz] = arr[:sz, c, :].T
        outs.append(o)
    return np.concatenate(outs, axis=0)


def kernel(**inputs) -> np.ndarray:
    nc = _get_nc()
    in_maps = make_in_maps(inputs)
    res = run_bass_kernel_spmd(nc, in_maps, core_ids=list(range(NCORES)))
    return assemble_out(res.results)


# revision 8
# speedup vs baseline: 1.3607x; 1.0220x over previous
"""Trainium2 Bass kernel for nn_DependencyEncoder (shift-reduce tree-LSTM).

Structure exploited (validated vs reference): transitions strictly alternate
shift/reduce, so the stack collapses to [tok0, v] and the module becomes 63
iterations of two tracking-LSTM steps plus one dependency composition:
  pair k:  track a on x=[tok_k, v, tok_0]
           track b on x=[tok_{k+1}, tok_k, v]
           head = tok_k (left arc) or v (right arc)
           gates = W_{l/r} @ head + W_c @ th_b ; v <- (h_j, c_j)
Output = v_h after the last pair.

Layout strategy (differs from the act-stationary baseline): everything is
GATE-MAJOR / WEIGHT-STATIONARY.  Weight tiles [K_feat<=128, M_gate<=128] are
the PE stationary operand; the moving operand is the activation [K_feat, B=48]
in fp16 (1 cycle/row in the cost model at any moving size, vs 4x penalty for
f32r under 256 rows).  Outputs land feature/gate-major [gate, batch] in PSUM,
so the LSTM elementwise runs directly on partition-aligned tiles and the next
iteration's matmul operands come out in the right orientation: NO transposes
anywhere in the loop.

Per iteration the PE sees only ~185 small matmuls (48-wide moving), the
sigmoid work is 6 Activation ops, and the cell math is ~20 DVE/Pool ops.
The tracking gates live on 64 partitions x 4 slots (i,f,2g,o) so every
cross-gate elementwise op is partition-aligned; composition gates live in a
[128, 15, 64] PSUM tile, slots (gate-block, H-chunk), H-chunks aligned with
the feature chunks of v / c_head.

Sharding: pure batch data-parallel, 384 rows -> 8 cores x 48 rows.
Masking (left/right arc) for tokens is folded on the host; only the v-side
masks run on-device.
"""
import numpy as np

import concourse.bacc as bacc
import concourse.bass as bass
import concourse.mybir as mybir
import concourse.tile as tile
from concourse.alu_op_type import AluOpType as alu
from concourse.bass import AP
from concourse.bass_utils import run_bass_kernel_spmd

AF = mybir.ActivationFunctionType
f32 = mybir.dt.float32
f16 = mybir.dt.float16

B_FULL, L, H, TD = 384, 64, 300, 64
NCORES = 8
B = B_FULL // NCORES          # 48 rows per core
K = L - 1                     # 63 pairs
CH = [(0, 128), (128, 128), (256, 44)]   # H chunks (offset, size)
NG = 4 * TD                   # 256 tracking gate columns


# --------------------------------------------------------------------------
# host-side weight preparation
# --------------------------------------------------------------------------
def _comp_wt(Wmat):
    """[5H, Kin] -> weight tiles [128, 3ci, 5g, 3co, 128] fp16.
    Gate-block order (fh, fc, i, u, o); u gets a real tanh table."""
    Wg = Wmat.astype(np.float32).reshape(5, H, -1)    # (i, o, fh, fc, u)
    Wg = np.stack([Wg[2], Wg[3], Wg[0], Wg[4], Wg[1]], axis=0)
    kin = Wmat.shape[1]
    out = np.zeros((128, 3, 5, 3, 128), np.float16)
    for ci, (offi, szi) in enumerate(CH):
        if offi >= kin:
            continue
        szi_eff = min(szi, kin - offi)
        for g in range(5):
            for co, (offo, szo) in enumerate(CH):
                blk = Wg[g, offo:offo + szo, offi:offi + szi_eff]   # [szo, szi]
                out[:szi_eff, ci, g, co, :szo] = blk.T
    return out


def _trk_wt(W_ih, W_hh):
    """Tracking weights -> [128, 3s, 3ci, 4slot, 64] fp16 and U [64, 4, 64].
    Slot order (i, f, o, g): torch row blocks (i, f, g, o) permuted so the
    sigmoid gates (i, f, o) are contiguous and g gets a real tanh table."""
    Wih = W_ih.astype(np.float32)        # [256, 900]
    Whh = W_hh.astype(np.float32)        # [256, 64]
    perm = [0, 1, 3, 2]                  # slot -> torch block
    wt = np.zeros((128, 3, 3, 4, 64), np.float16)
    for s in range(3):
        for ci, (offi, szi) in enumerate(CH):
            for slot in range(4):
                b = perm[slot]
                rows = Wih[b * 64:(b + 1) * 64,
                           s * H + offi: s * H + offi + szi]   # [64, szi]
                wt[:szi, s, ci, slot, :] = rows.T
    wu = np.zeros((64, 4, 64), np.float16)
    for slot in range(4):
        b = perm[slot]
        wu[:, slot, :] = Whh[b * 64:(b + 1) * 64, :].T
    return wt, wu


def _prep_host(inputs):
    W_c = np.asarray(inputs["W_c"], np.float32)
    Uh_w = np.asarray(inputs["Uh_w"], np.float32)
    Ul_w = np.asarray(inputs["Ul_w"], np.float32)
    Ur_w = np.asarray(inputs["Ur_w"], np.float32)
    wl = _comp_wt(Uh_w + Ul_w)
    wr = _comp_wt(Uh_w + Ur_w)
    wcc_full = _comp_wt(W_c)             # Kin=64 -> only ci=0 rows used
    wcc = np.ascontiguousarray(wcc_full[:64, 0])     # [64, 5, 3, 128]
    wtrk, whh = _trk_wt(np.asarray(inputs["W_ih"]), np.asarray(inputs["W_hh"]))
    return dict(wl=wl, wr=wr, wcc=wcc, wtrk=wtrk, whh=whh)


# --------------------------------------------------------------------------
# device program
# --------------------------------------------------------------------------
_CACHED_NC = None


def _build_nc():
    nc = bacc.Bacc("TRN2", target_bir_lowering=False)
    tokh_d = nc.dram_tensor("tokh", [128, L, 3, B], f16, kind="ExternalInput")
    tokl_d = nc.dram_tensor("tokl", [128, K, 3, B], f16, kind="ExternalInput")
    tokcl_d = nc.dram_tensor("tokcl", [128, K, 3, B], f32, kind="ExternalInput")
    tokc0_d = nc.dram_tensor("tokc0", [128, 3, B], f32, kind="ExternalInput")
    wl_d = nc.dram_tensor("wl", [128, 3, 5, 3, 128], f16, kind="ExternalInput")
    wr_d = nc.dram_tensor("wr", [128, 3, 5, 3, 128], f16, kind="ExternalInput")
    wcc_d = nc.dram_tensor("wcc", [64, 5, 3, 128], f16, kind="ExternalInput")
    wtrk_d = nc.dram_tensor("wtrk", [128, 3, 3, 4, 64], f16, kind="ExternalInput")
    whh_d = nc.dram_tensor("whh", [64, 4, 64], f16, kind="ExternalInput")
    mrf_d = nc.dram_tensor("mrf", [K, B], f32, kind="ExternalInput")
    mrh_d = nc.dram_tensor("mrh", [K, B], f16, kind="ExternalInput")
    th0_d = nc.dram_tensor("th0", [TD, B], f16, kind="ExternalInput")
    tc0_d = nc.dram_tensor("tc0", [TD, B], f32, kind="ExternalInput")
    out_d = nc.dram_tensor("out", [128, 3, B], f32, kind="ExternalOutput")

    mm = None  # set below

    with tile.TileContext(nc) as tc_:
        with (
            tc_.tile_pool(name="sg", bufs=1) as sg,
            tc_.tile_pool(name="st", bufs=2) as st,
            tc_.tile_pool(name="rot", bufs=2) as rot,
            tc_.tile_pool(name="psT", bufs=2, space="PSUM") as psT,
            tc_.tile_pool(name="psC", bufs=2, space="PSUM") as psC,
        ):
            # ---------------- resident tiles ----------------
            tokh = sg.tile([128, L, 3, B], f16)
            tokl = sg.tile([128, K, 3, B], f16)
            tokcl = sg.tile([128, K, 3, B], f32)
            tokc0 = sg.tile([128, 3, B], f32)
            wl = sg.tile([128, 3, 5, 3, 128], f16)
            wr = sg.tile([128, 3, 5, 3, 128], f16)
            wcc = sg.tile([64, 5, 3, 128], f16)
            wtrk = sg.tile([128, 3, 3, 4, 64], f16)
            whh = sg.tile([64, 4, 64], f16)
            mrf = sg.tile([128, K, B], f32)
            mrh = sg.tile([128, K, B], f16)
            th0 = sg.tile([TD, B], f16)
            tc0 = sg.tile([TD, B], f32)

            # weights + state first, then tokens in groups so iter-0 deps
            # land early
            nc.sync.dma_start(wl[:], wl_d[:])
            nc.sync.dma_start(wr[:], wr_d[:])
            nc.sync.dma_start(wtrk[:], wtrk_d[:])
            nc.sync.dma_start(whh[:], whh_d[:])
            nc.sync.dma_start(wcc[:], wcc_d[:])
            for dst, srcd in ((mrf, mrf_d), (mrh, mrh_d)):
                bsrc = AP(tensor=srcd, offset=0, ap=[[0, 128], [B, K], [1, B]])
                nc.sync.dma_start(dst[:], bsrc)
            nc.sync.dma_start(th0[:], th0_d[:])
            nc.sync.dma_start(tc0[:], tc0_d[:])
            nc.sync.dma_start(tokc0[:], tokc0_d[:])
            NGRP = 8
            for g in range(NGRP):
                lo = g * (L // NGRP)
                hi = lo + (L // NGRP)
                nc.sync.dma_start(tokh[:, lo:hi], tokh_d[:, lo:hi])
                khi = min(hi, K)
                if lo < K:
                    nc.sync.dma_start(tokl[:, lo:khi], tokl_d[:, lo:khi])
                    nc.sync.dma_start(tokcl[:, lo:khi], tokcl_d[:, lo:khi])

            mm = nc.tensor.matmul

            # zero the never-written partition rows (44:128) of both comp
            # psum buffers once, so sigmoid reads are finite there forever
            for _b in range(2):
                cpz = psC.tile([128, 15, 64], f32, tag="cp")
                nc.vector.memset(cpz[44:128, :, :], 0.0)

            def mask_h(k):
                return mrh[:, k, :].unsqueeze(1).broadcast_to((128, 3, B))

            def mask_f(k):
                return mrf[:, k, :].unsqueeze(1).broadcast_to((128, 3, B))

            # ---- initial state: v = tok0, hr_0 = mr_0*tok0, ch_0 ----
            vb = tokh[:, 0]              # [128, 3, B] fp16 (h part of v)
            vc = tokc0                   # [128, 3, B] f32  (c part of v)
            hr = rot.tile([128, 3, B], f16, tag="hr")
            nc.vector.tensor_tensor(hr[:], tokh[:, 0], mask_h(0), alu.mult)
            mrvc = rot.tile([128, 3, B], f32, tag="mrvc")
            nc.gpsimd.tensor_tensor(mrvc[:], tokc0[:], mask_f(0), alu.mult)
            ch = rot.tile([128, 3, B], f32, tag="ch")
            nc.gpsimd.tensor_tensor(ch[:], tokcl[:, 0], mrvc[:], alu.add)
            thb = th0
            tc_st = tc0

            def track_mms(ps, tok_blocks, v_in, th_in):
                """Emit one tracking step's matmuls into psum ps [64, 4, 48].
                tok_blocks: list of (block_idx, token_ap_3chunk) emitted
                before the v part; v enters as block v_blk."""
                for slot in range(4):
                    mm(ps[:, slot, :], whh[:, slot, :], th_in[:],
                       start=True, stop=False)
                for blk, tok in tok_blocks:
                    for slot in range(4):
                        for ci, (offi, szi) in enumerate(CH):
                            mm(ps[:, slot, :], wtrk[:szi, blk, ci, slot, :],
                               tok[:szi, ci, :], start=False, stop=False)
                return ps

            def track_v_mms(ps, v_blk, v_in):
                for slot in range(4):
                    for ci, (offi, szi) in enumerate(CH):
                        mm(ps[:, slot, :], wtrk[:szi, v_blk, ci, slot, :],
                           v_in[:szi, ci, :], start=False,
                           stop=(ci == 2))
                return ps

            def track_tail(ps, tc_in):
                """sigmoid/tanh + LSTM cell; returns (thb fp16 [64,48], tc).
                Slots (i, f, o, g): sigmoid the first three in one op, real
                tanh for g and for tc'."""
                sa = st.tile([64, 3, B], f32, tag="sa")
                nc.scalar.activation(sa[:], ps[:, 0:3, :], AF.Sigmoid)
                tg = st.tile([64, B], f32, tag="tg")
                nc.scalar.activation(tg[:], ps[:, 3, :], AF.Tanh)
                si, sf, so = sa[:, 0, :], sa[:, 1, :], sa[:, 2, :]
                p = st.tile([64, B], f32, tag="p")
                nc.vector.tensor_tensor(p[:], si, tg[:], alu.mult)
                r = st.tile([64, B], f32, tag="r")
                nc.gpsimd.tensor_tensor(r[:], sf, tc_in[:], alu.mult)
                tco = st.tile([64, B], f32, tag="tc")
                nc.vector.tensor_tensor(tco[:], p[:], r[:], alu.add)
                tt = st.tile([64, B], f32, tag="tt")
                nc.scalar.activation(tt[:], tco[:], AF.Tanh)
                tho = st.tile([64, B], f16, tag="th")
                nc.vector.tensor_tensor(tho[:], so, tt[:], alu.mult)
                return tho, tco

            for k in range(K):
                # ---------- PE: ordered so stalling mms come late ----------
                ta = psT.tile([64, 4, B], f32, tag="ta")
                tb = psT.tile([64, 4, B], f32, tag="ta")
                cp = psC.tile([128, 15, 64], f32, tag="cp")

                # step a: U first (th ready), then token blocks
                track_mms(ta, [(0, tokh[:, k]), (2, tokh[:, 0])], None, thb)
                # step b token parts (U comes after th_a)
                for slot in range(4):
                    for blk, tok in ((0, tokh[:, k + 1]), (1, tokh[:, k])):
                        for ci, (offi, szi) in enumerate(CH):
                            mm(tb[:, slot, :], wtrk[:szi, blk, ci, slot, :],
                               tok[:szi, ci, :],
                               start=(blk == 0 and ci == 0), stop=False)
                # step a v part (stalls until v ready; ahead of comp-l so the
                # sigmoid_a chain isn't queued behind 45 composition matmuls)
                track_v_mms(ta, 1, vb)
                # comp left (host-masked tokens; fills PE during track-a tail)
                for g in range(5):
                    for co, (offo, szo) in enumerate(CH):
                        slot = g * 3 + co
                        for ci, (offi, szi) in enumerate(CH):
                            mm(cp[:szo, slot, :B], wl[:szi, ci, g, co, :szo],
                               tokl[:szi, k, ci, :],
                               start=(ci == 0), stop=False)
                # step b v part
                for slot in range(4):
                    for ci, (offi, szi) in enumerate(CH):
                        mm(tb[:, slot, :], wtrk[:szi, 2, ci, slot, :],
                           vb[:szi, ci, :], start=False, stop=False)
                # comp right (stalls until hr ready)
                for g in range(5):
                    for co, (offo, szo) in enumerate(CH):
                        slot = g * 3 + co
                        for ci, (offi, szi) in enumerate(CH):
                            mm(cp[:szo, slot, :B], wr[:szi, ci, g, co, :szo],
                               hr[:szi, ci, :], start=False, stop=False)

                # ---------- track a tail ----------
                thb_a, tc_a = track_tail(ta, tc_st)

                # step b U (waits th_a)
                for slot in range(4):
                    mm(tb[:, slot, :], whh[:, slot, :], thb_a[:],
                       start=False, stop=True)

                # ---------- track b tail ----------
                thb_b, tc_b = track_tail(tb, tc_a)

                # comp W_c (waits th_b)
                for g in range(5):
                    for co, (offo, szo) in enumerate(CH):
                        slot = g * 3 + co
                        mm(cp[:szo, slot, :B], wcc[:, g, co, :szo],
                           thb_b[:], start=False, stop=True)

                # ---------- composition tail ----------
                # sigmoid (fh, fc, i) first so the DVE chain starts early,
                # then tanh(u), then sigmoid(o)
                sc = rot.tile([128, 9, B], f32, tag="sc")
                nc.scalar.activation(sc[:], cp[:, 0:9, :B], AF.Sigmoid)
                tu = rot.tile([128, 3, B], f32, tag="tu")
                nc.scalar.activation(tu[:], cp[:, 9:12, :B], AF.Tanh)
                soc = rot.tile([128, 3, B], f32, tag="soc")
                nc.scalar.activation(soc[:], cp[:, 12:15, :B], AF.Sigmoid)
                SFH = sc[:, 0:3, :]
                SFC = sc[:, 3:6, :]
                SI = sc[:, 6:9, :]
                s2 = rot.tile([128, 3, B], f32, tag="s2")
                nc.vector.tensor_tensor(s2[:], SFH, SFC, alu.add)
                t3 = rot.tile([128, 3, B], f32, tag="t3")
                nc.vector.tensor_tensor(t3[:], s2[:], ch[:], alu.mult)
                pu = rot.tile([128, 3, B], f32, tag="pu")
                nc.gpsimd.tensor_tensor(pu[:], SI, tu[:], alu.mult)
                cj = rot.tile([128, 3, B], f32, tag="vc")
                nc.vector.tensor_tensor(cj[:], t3[:], pu[:], alu.add)
                tcj = rot.tile([128, 3, B], f32, tag="tcj")
                nc.scalar.activation(tcj[:], cj[:], AF.Tanh)
                hj = rot.tile([128, 3, B], f16, tag="vb")
                nc.vector.tensor_tensor(hj[:], tcj[:], soc[:], alu.mult)
                if k + 1 < K:
                    mso = rot.tile([128, 3, B], f32, tag="mso")
                    nc.gpsimd.tensor_tensor(mso[:], soc[:], mask_f(k + 1),
                                            alu.mult)
                    hr = rot.tile([128, 3, B], f16, tag="hr")
                    nc.gpsimd.tensor_tensor(hr[:], mso[:], tcj[:], alu.mult)
                    mrvc = rot.tile([128, 3, B], f32, tag="mrvc")
                    nc.gpsimd.tensor_tensor(mrvc[:], cj[:], mask_f(k + 1),
                                            alu.mult)
                    ch = rot.tile([128, 3, B], f32, tag="ch")
                    nc.gpsimd.tensor_tensor(ch[:], tokcl[:, k + 1], mrvc[:],
                                            alu.add)
                else:
                    # final output in f32
                    hjf = rot.tile([128, 3, B], f32, tag="hjf")
                    nc.vector.tensor_tensor(hjf[:], tcj[:], soc[:], alu.mult)
                    nc.sync.dma_start(out_d[:], hjf[:])

                vb = hj
                vc = cj
                thb = thb_b
                tc_st = tc_b

    nc.compile()
    return nc


def _get_nc():
    global _CACHED_NC
    if _CACHED_NC is None:
        _CACHED_NC = _build_nc()
    return _CACHED_NC


# --------------------------------------------------------------------------
# host-side input marshaling
# --------------------------------------------------------------------------
def make_in_maps(inputs):
    seq = np.asarray(inputs["sequence"], np.float32)
    tr = np.asarray(inputs["transitions"])
    th0 = np.asarray(inputs["th0"], np.float32)
    tc0 = np.asarray(inputs["tc0"], np.float32)
    wts = _prep_host(inputs)

    in_maps = []
    for i in range(NCORES):
        s = slice(i * B, (i + 1) * B)
        sq = seq[s]                                  # [B, L, 600]
        toh = sq[:, :, :H]                           # [B, L, 300]
        toc = sq[:, :, H:]
        is_left = (tr[s, 1::2].T == 2).astype(np.float32)[:K]   # [K, B]
        mr = 1.0 - is_left                                       # right mask

        tokh = np.zeros((128, L, 3, B), np.float16)
        tokl = np.zeros((128, K, 3, B), np.float16)
        tokcl = np.zeros((128, K, 3, B), np.float32)
        for c, (off, sz) in enumerate(CH):
            tokh[:sz, :, c] = toh[:, :, off:off + sz].transpose(2, 1, 0)
            tokl[:sz, :, c] = (toh[:, :K, off:off + sz]
                               * is_left.T[:, :, None]).transpose(2, 1, 0)
            tokcl[:sz, :, c] = (toc[:, :K, off:off + sz]
                                * is_left.T[:, :, None]).transpose(2, 1, 0)
        tokc0 = np.zeros((128, 3, B), np.float32)
        for c, (off, sz) in enumerate(CH):
            tokc0[:sz, c] = toc[:, 0, off:off + sz].T

        in_maps.append(dict(
            tokh=tokh, tokl=tokl, tokcl=tokcl, tokc0=tokc0,
            wl=wts["wl"], wr=wts["wr"], wcc=wts["wcc"],
            wtrk=wts["wtrk"], whh=wts["whh"],
            mrf=np.ascontiguousarray(mr),
            mrh=np.ascontiguousarray(mr.astype(np.float16)),
            th0=np.ascontiguousarray(th0[s].T.astype(np.float16)),
            tc0=np.ascontiguousarray(tc0[s].T),
        ))
    return in_maps


def assemble_out(res_list):
    outs = []
    for r in res_list:
        arr = r["out"]                       # [128, 3, B]
        o = np.empty((B, H), np.float32)
        for c, (off, sz) in enumerate(CH):
            o[:, off:off + sz] = arr[:sz, c, :].T
        outs.append(o)
    return np.concatenate(outs, axis=0)


def kernel(**inputs) -> np.ndarray:
    nc = _get_nc()
    in_maps = make_in_maps(inputs)
    res = run_bass_kernel_spmd(nc, in_maps, core_ids=list(range(NCORES)))
    return assemble_out(res.results)
